# revision 1
# baseline (speedup 1.0000x reference)
"""Trainium2 Bass kernel for nn_Interaction_GraphConvolution (GNN message passing).

Math (N=2048, F_IN=128, F=64):
    H = X @ W + b                                      # [N, F]
    out[j,f] = sum_k mf[j,k] * H[k,f] * G_k[j,f]
    G_k[j,f] = sum_i A[j,i] * H[i,f] * mh[i,k]         # one [N,N]@[N,F] matmul per k

Sharding: k axis split across 8 cores (256 k's each). Each core holds A and H
(replicated) plus its mh/mf column shards, computes the partial sum over its k
slice, and the host adds the 8 partials.

Per-core schedule (PE kept ~pure matmul; transposes/broadcasts on DMA):
  - A^T tiles: DMA A row block -> cast bf16 -> one 3-D xbar DMA transpose per
    half block into at[jt][p, it, q] = A^T[it*128+p, jt*128+q].
  - X^T for H = X@W+b the same way; bias added via ones-row matmul.
  - Hrow broadcast hk[p,(k,f)] = H[k,f]: step-0 partition-broadcast DMA from
    the on-device Hsh scratch in DRAM.
  - k's processed in chunks of KB=8 (512 matmul cols = 8 k x 64 f):
      R[i,(k,f)] = H[i,f]*mh[i,k]     one DVE op w/ step-0 broadcast APs
      G = A @ R                       16 accumulating bf16 matmuls -> fp32 psum
      t1 = G*mf_b; t1 *= hk; acc += t1    3 DVE ops (2 in place)
  - Final: reduce acc over the 8 k-chunk slots, DMA out.
"""

import numpy as np

import concourse.bacc as bacc
import concourse.mybir as mybir
from concourse.tile import TileContext
from concourse.masks import make_identity
from concourse.bass_utils import run_bass_kernel_spmd

N = 2048
FIN = 128
F = 64
P = 128
NCORES = 8
KSH = N // NCORES          # 256 k's per core
KB = 8                     # k's per chunk (512 matmul cols)
NKB = KSH // KB            # 32 chunks per core
NIT = N // P               # 16 i tiles
NJT = N // P               # 16 j tiles
NCOL = KB * F              # 512

_CACHE = {}


def _build():
    dt = mybir.dt
    nc = bacc.Bacc("TRN2")

    x_in = nc.declare_dram_parameter("x", [N, FIN], dt.float32, isOutput=False)
    xs_in = nc.declare_dram_parameter("xs", [KSH, FIN], dt.float32, isOutput=False)
    w_in = nc.declare_dram_parameter("w", [FIN, F], dt.float32, isOutput=False)
    b_in = nc.declare_dram_parameter("b", [1, F], dt.float32, isOutput=False)
    a_in = nc.declare_dram_parameter("a", [N, N], dt.float32, isOutput=False)
    mh_in = nc.declare_dram_parameter("mh", [N, KSH], dt.float32, isOutput=False)
    mf_in = nc.declare_dram_parameter("mf", [N, KSH], dt.float32, isOutput=False)
    out_p = nc.declare_dram_parameter("out_p", [N, F], dt.float32, isOutput=True)

    hsh_dram = nc.dram_tensor("hsh_flat", [1, KSH * F], dt.float32)

    with TileContext(nc) as tc:
        with (
            tc.tile_pool(name="const", bufs=1) as cpool,
            tc.tile_pool(name="stage", bufs=2) as stage,
            tc.tile_pool(name="work", bufs=1) as work,
            tc.tile_pool(name="rp", bufs=2) as rp,
            tc.tile_pool(name="tmp", bufs=3) as tmp,
            tc.tile_pool(name="hk", bufs=3) as hkp,
            tc.tile_pool(name="psg", bufs=6, space="PSUM") as psg,
            tc.tile_pool(name="psm", bufs=2, space="PSUM") as psm,
        ):
            ones = cpool.tile([1, P], dt.float32)
            nc.any.memset(ones, 1.0)
            ident = cpool.tile([P, P], dt.bfloat16)
            make_identity(nc, ident)

            # ---- weights / bias ----
            w_sb = cpool.tile([FIN, F], dt.float32)
            nc.sync.dma_start(out=w_sb, in_=w_in[:, :])
            w_bf = cpool.tile([FIN, F], dt.bfloat16)
            nc.any.tensor_copy(out=w_bf, in_=w_sb)
            b_sb = cpool.tile([1, F], dt.float32)
            nc.sync.dma_start(out=b_sb, in_=b_in[:, :])

            def h_tile(src_ap, dst_sb, tag):
                """dst_sb[128, F] = (src_rows @ W + b) for a 128-row block."""
                x_st = stage.tile([P, FIN], dt.float32, tag="xst", name="xst")
                nc.sync.dma_start(out=x_st, in_=src_ap)
                x_bf = stage.tile([P, FIN], dt.bfloat16, tag="xbf", name="xbf")
                nc.any.tensor_copy(out=x_bf, in_=x_st)
                xt_ps = psm.tile([P, P], dt.bfloat16, tag="m", name="xtps")
                nc.tensor.transpose(xt_ps, x_bf, ident)
                xt_bf = stage.tile([P, P], dt.bfloat16, tag="xtbf", name="xtbf")
                nc.any.tensor_copy(out=xt_bf, in_=xt_ps)
                h_ps = psm.tile([P, F], dt.float32, tag="m", name="hps")
                nc.tensor.matmul(h_ps, xt_bf, w_bf, start=True, stop=False)
                nc.tensor.matmul(h_ps, ones, b_sb, start=False, stop=True)
                nc.any.tensor_copy(out=dst_sb, in_=h_ps)

            # ---- H = X @ W + b  (16 tiles, stays in SBUF) ----
            h_sb = [cpool.tile([P, F], dt.float32, tag=f"h{i}", name=f"h{i}")
                    for i in range(NIT)]
            for i in range(NIT):
                h_tile(x_in[i * P:(i + 1) * P, :], h_sb[i], f"h{i}")

            # ---- Hsh rows (this core's k shard) -> DRAM scratch ----
            for t in range(KSH // P):
                hs_sb = stage.tile([P, F], dt.float32, tag="hs", name="hs")
                h_tile(xs_in[t * P:(t + 1) * P, :], hs_sb, "hs")
                nc.sync.dma_start(
                    out=hsh_dram[0:1, t * P * F:(t + 1) * P * F], in_=hs_sb
                )

            # ---- mh (bf16) / mf (fp32) shards ----
            mh_sb = []
            mf_sb = []
            for i in range(NIT):
                m_st = stage.tile([P, KSH], dt.float32, tag="mst", name="mst")
                nc.sync.dma_start(out=m_st, in_=mh_in[i * P:(i + 1) * P, :])
                mh_t = work.tile([P, KSH], dt.bfloat16, tag=f"mh{i}", name=f"mh{i}")
                nc.any.tensor_copy(out=mh_t, in_=m_st)
                mh_sb.append(mh_t)
                mf_t = work.tile([P, KSH], dt.float32, tag=f"mf{i}", name=f"mf{i}")
                nc.sync.dma_start(out=mf_t, in_=mf_in[i * P:(i + 1) * P, :])
                mf_sb.append(mf_t)

            # ---- A^T tiles: at[jt][p, it, q] = A[jt*128+q, it*128+p] ----
            at = [work.tile([P, NIT, P], dt.bfloat16, tag=f"at{j}", name=f"at{j}")
                  for j in range(NJT)]
            NQ = 2
            for jt in range(NJT):
                for q in range(NQ):
                    cols = N // NQ
                    a_st = stage.tile([P, cols], dt.float32, tag="ast",
                                      name="ast", bufs=4)
                    nc.sync.dma_start(
                        out=a_st,
                        in_=a_in[jt * P:(jt + 1) * P,
                                 q * cols:(q + 1) * cols],
                    )
                    a_bf = stage.tile([P, cols], dt.bfloat16, tag="abf",
                                      name="abf", bufs=4)
                    nc.any.tensor_copy(out=a_bf, in_=a_st)
                    for w8 in range(NIT // NQ):
                        it = q * (NIT // NQ) + w8
                        t_ps = psm.tile([P, P], dt.bfloat16, tag="m",
                                        name="tps")
                        nc.tensor.transpose(
                            t_ps, a_bf[:, w8 * P:(w8 + 1) * P], ident
                        )
                        nc.any.tensor_copy(out=at[jt][:, it, :], in_=t_ps)

            # ---- accumulators ----
            acc = [work.tile([P, NCOL], dt.float32, tag=f"acc{j}", name=f"acc{j}")
                   for j in range(NJT)]
            for j in range(NJT):
                nc.any.memset(acc[j], 0.0)

            # ---- main loop over k chunks ----
            for kb in range(NKB):
                # hk[p,(k,f)] = Hsh[kb*KB+k, f] via partition-broadcast DMA
                hk = hkp.tile([P, NCOL], dt.float32, tag="hk", name="hk")
                nc.sync.dma_start(
                    out=hk,
                    in_=hsh_dram[0:1, kb * NCOL:(kb + 1) * NCOL]
                    .partition_broadcast(P),
                )

                # R tiles for this chunk
                r_kb = []
                for it in range(NIT):
                    r_t = rp.tile([P, NCOL], dt.bfloat16, tag=f"r{it}",
                                  name=f"r{it}")
                    h_b = h_sb[it][:, :].unsqueeze(1).to_broadcast([P, KB, F])
                    mh_b = (
                        mh_sb[it][:, kb * KB:(kb + 1) * KB]
                        .unsqueeze(2)
                        .to_broadcast([P, KB, F])
                    )
                    r_view = r_t[:, :].rearrange("p (k f) -> p k f", k=KB)
                    nc.vector.tensor_mul(r_view, h_b, mh_b)
                    r_kb.append(r_t)

                for jt in range(NJT):
                    g_ps = psg.tile([P, NCOL], dt.float32, tag="g", name="g")
                    for it in range(NIT):
                        nc.tensor.matmul(
                            g_ps,
                            at[jt][:, it, :],
                            r_kb[it],
                            start=(it == 0),
                            stop=(it == NIT - 1),
                        )
                    # epilogue: acc[jt] += g * mf[:,k] * hk
                    # mf scaling on the (otherwise idle) scalar engine,
                    # one per k chunk with a per-partition scale vector
                    t1 = tmp.tile([P, NCOL], dt.float32, tag="t1", name="t1")
                    for kc in range(KB):
                        nc.scalar.activation(
                            out=t1[:, kc * F:(kc + 1) * F],
                            in_=g_ps[:, kc * F:(kc + 1) * F],
                            func=mybir.ActivationFunctionType.Copy,
                            scale=mf_sb[jt][:, kb * KB + kc:kb * KB + kc + 1],
                        )
                    nc.vector.tensor_mul(t1, t1, hk)
                    nc.vector.tensor_add(acc[jt], acc[jt], t1)

            # ---- finale: reduce k-chunk slots, store ----
            for jt in range(NJT):
                red = stage.tile([P, F], dt.float32, tag="red", name="red")
                nc.vector.tensor_reduce(
                    red,
                    acc[jt][:, :].rearrange("p (k f) -> p f k", k=KB),
                    axis=mybir.AxisListType.X,
                    op=mybir.AluOpType.add,
                )
                nc.sync.dma_start(out=out_p[jt * P:(jt + 1) * P, :], in_=red)

    nc.finalize()
    return nc


def _get_nc():
    if "nc" not in _CACHE:
        _CACHE["nc"] = _build()
    return _CACHE["nc"]


def _in_maps(node_features, adjacency_matrix, mask_father, mask_hadamard,
             weight, bias):
    x = np.ascontiguousarray(node_features, dtype=np.float32)
    a = np.ascontiguousarray(adjacency_matrix, dtype=np.float32)
    mf = np.ascontiguousarray(mask_father, dtype=np.float32)
    mh = np.ascontiguousarray(mask_hadamard, dtype=np.float32)
    w = np.ascontiguousarray(weight, dtype=np.float32)
    b = np.ascontiguousarray(bias, dtype=np.float32).reshape(1, F)
    maps = []
    for c in range(NCORES):
        s = slice(c * KSH, (c + 1) * KSH)
        maps.append({
            "x": x,
            "xs": np.ascontiguousarray(x[s, :]),
            "w": w,
            "b": b,
            "a": a,
            "mh": np.ascontiguousarray(mh[:, s]),
            "mf": np.ascontiguousarray(mf[:, s]),
        })
    return maps


def run_spmd(inputs, **kw):
    """Run the SPMD kernel; returns (summed_output, BassKernelResults)."""
    nc = _get_nc()
    maps = _in_maps(**inputs)
    res = run_bass_kernel_spmd(nc, maps, list(range(NCORES)), **kw)
    out = np.zeros((N, F), dtype=np.float32)
    for c in range(NCORES):
        out += res.results[c]["out_p"]
    return out, res


def kernel(node_features, adjacency_matrix, mask_father, mask_hadamard,
           weight, bias):
    out, _ = run_spmd(dict(
        node_features=node_features,
        adjacency_matrix=adjacency_matrix,
        mask_father=mask_father,
        mask_hadamard=mask_hadamard,
        weight=weight,
        bias=bias,
    ))
    return out

